# revision 17
# baseline (speedup 1.0000x reference)
"""Trainium2 Bass kernel for nn_CMR_59931973648949 (gnn_message_passing).

Contract: kernel(**inputs) takes FULL unsharded numpy inputs and returns the
FULL [16, 1024] output. Data-parallel over batch across 8 cores (2 samples
per core, weights replicated). All weights are host-packed partition-major
([128, F]) for max-bandwidth DMAs; the two local samples are batched through
the shared heavy matmuls (feat_v, q/u0/u1 projections); the big matmuls run
as float32r (full PE rate at free-dim >= 256).

Math per sample (refactored; see git history for derivation):
  scl[n] = mean(norm_w)/max(||visf[:,n]||,1e-12)   (folded into feat_v scale)
  feat_v = (visf.T * scl) @ W_v.T ; used only via feat_vT
  q/u0/u1 from node/relate reps with WnT=W_node.T/sqrt(DV),
      WA0/1=W_rel.T@W_e[:, :DV | DV:]/sqrt(DE)
  find = softmax(mask(q @ feat_vT)) * node_mask
  ea_r = sigmoid(A0[r,:] bcast + A1T[:,r]) * relation_mask
  g_findT = find.T-gather via GT (folds valid*relate_mask*onehot(obj))
  h[r,:] = g_find[r,:] @ ea_r ; find2T = findT + h.T @ ST (onehot(subj))
  fa = rowmax(find2T); fa /= max(max(fa),1); fa = fa*bm + (1-bm)*1e-7
  mem = visf @ fa ; out = mem @ W_out.T + b_out
"""

import numpy as np

import concourse.bass as bass
import concourse.tile as tile
from concourse import bacc, mybir
from concourse.bass_utils import run_bass_kernel_spmd

P = 128
B, K, R, N = 16, 12, 12, 64
DW, DV, DVIS, DE, DC = 512, 512, 2048, 512, 1024
NCORES = 8
S = B // NCORES  # samples per core = 2
N2 = S * N  # 128: both samples' boxes side by side
K2 = S * K  # 24

F32 = mybir.dt.float32
F32R = mybir.dt.float32r
USE_F32R = True

# smalls packing column offsets (per sample, [64, SMALLS_F])
_SM_RM = 0          # rmask      [64, 64]
_SM_BM = 64         # bmmul      [12, 64]
_SM_BA = 128        # bmadd      [12, 64]
_SM_GT = 192        # GT         [12, 12]
_SM_ST = 204        # ST         [12, 12]
_SM_NM = 216        # nmcol      [12, 1]
_SM_FM = 217        # famul      [1, 64]
_SM_FA = 281        # faadd      [1, 64]
SMALLS_F = 352

_cache = {}


def _pack(a):
    """[(o*128), F] row-major -> [128, o*F] partition-major."""
    o = a.shape[0] // P
    return np.ascontiguousarray(
        a.reshape(o, P, a.shape[1]).transpose(1, 0, 2).reshape(P, -1)
    )


def _r(ap):
    return ap.bitcast(F32R) if USE_F32R else ap


def build_nc():
    nc = bacc.Bacc(num_devices=NCORES)

    FR = F32R if USE_F32R else F32
    d_visf2 = nc.declare_dram_parameter("visf2", [P, 16 * N2], FR, isOutput=False)
    d_reps = nc.declare_dram_parameter("reps", [P, 2 * 4 * K2], F32, isOutput=False)
    d_WvT = nc.declare_dram_parameter("WvT", [P, 16 * DV], FR, isOutput=False)
    d_WnT = nc.declare_dram_parameter("WnT", [P, 4 * DV], F32, isOutput=False)
    d_WA0 = nc.declare_dram_parameter("WA0", [P, 4 * DV], F32, isOutput=False)
    d_WA1 = nc.declare_dram_parameter("WA1", [P, 4 * DV], F32, isOutput=False)
    d_WoT = nc.declare_dram_parameter("WoT", [P, 16 * DC], FR, isOutput=False)
    d_bout = nc.declare_dram_parameter("bout", [S, DC], F32, isOutput=False)
    d_smalls = nc.declare_dram_parameter("smalls", [S, N, SMALLS_F], F32, isOutput=False)
    d_I128 = nc.declare_dram_parameter("I128", [P, P], F32, isOutput=False)
    d_out = nc.declare_dram_parameter("out", [S, DC], F32, isOutput=True)

    with tile.TileContext(nc) as tc:
        with (
            tc.tile_pool(name="singles", bufs=1) as singles,
            tc.tile_pool(name="ps", bufs=2) as ps,
            tc.tile_pool(name="psum", bufs=8, space="PSUM") as psum,
        ):
            # ---- DMAs on the critical path first (SP queue runs in order) ----
            I128_sb = singles.tile([P, P], F32)
            nc.sync.dma_start(out=I128_sb[:], in_=d_I128[:])
            reps_sb = singles.tile([P, 2, 4, K2], F32)
            nc.sync.dma_start(
                out=reps_sb[:], in_=d_reps[:].rearrange("p (t o k) -> p t o k", t=2, o=4)
            )
            visf2_sb = singles.tile([P, 16, N2], F32R if USE_F32R else F32)
            nc.sync.dma_start(
                out=visf2_sb[:], in_=d_visf2[:].rearrange("p (o n) -> p o n", o=16)
            )
            visf2_f = visf2_sb[:].bitcast(F32)  # rounded bits, read as f32
            smalls_sb = []
            for s in range(S):
                sm = ps.tile([N, SMALLS_F], F32, name=f"smalls{s}", tag=f"smalls{s}")
                nc.sync.dma_start(out=sm[:], in_=d_smalls[s])
                smalls_sb.append(sm)
            WvT_sb = singles.tile([P, 16, DV], F32R if USE_F32R else F32)
            for g in range(4):
                nc.sync.dma_start(
                    out=WvT_sb[:, 4 * g : 4 * g + 4, :],
                    in_=d_WvT[:, 4 * g * DV : 4 * (g + 1) * DV].rearrange(
                        "p (o d) -> p o d", o=4
                    ),
                )
            WnT_sb = singles.tile([P, 4, DV], F32)
            nc.sync.dma_start(
                out=WnT_sb[:], in_=d_WnT[:].rearrange("p (o d) -> p o d", o=4)
            )
            WA0_sb = singles.tile([P, 4, DV], F32)
            nc.sync.dma_start(
                out=WA0_sb[:], in_=d_WA0[:].rearrange("p (o d) -> p o d", o=4)
            )
            WA1_sb = singles.tile([P, 4, DV], F32)
            nc.sync.dma_start(
                out=WA1_sb[:], in_=d_WA1[:].rearrange("p (o d) -> p o d", o=4)
            )

            ones_col = singles.tile([P, 1], F32)
            nc.vector.memset(ones_col[:], 1.0)
            ones_1xP = singles.tile([1, P], F32)
            nc.vector.memset(ones_1xP[:], 1.0)
            mem2_sb = singles.tile([P, 16, S], F32)

            nrep2 = reps_sb[:, 0]  # [P, 4, 24]
            rrep2 = reps_sb[:, 1]

            # ---- shared: column norms -> scl (both samples at once) ----
            sq_sb = singles.tile([P, 16, N2], F32)
            nc.vector.tensor_tensor(
                out=sq_sb[:], in0=visf2_f, in1=visf2_f,
                op=mybir.AluOpType.mult,
            )
            presum = singles.tile([P, N2], F32)
            nc.vector.tensor_reduce(
                out=presum[:],
                in_=sq_sb[:].rearrange("p o n -> p n o"),
                axis=mybir.AxisListType.X,
                op=mybir.AluOpType.add,
            )
            sqn_ps = psum.tile([N2, 1], F32, tag="ps")
            nc.tensor.matmul(
                out=sqn_ps[:], lhsT=presum[:], rhs=ones_col[:], start=True, stop=True
            )
            scl = singles.tile([N2, 1], F32)
            nc.scalar.sqrt(out=scl[:], in_=sqn_ps[:])
            nc.vector.tensor_scalar_max(out=scl[:], in0=scl[:], scalar1=1e-12)
            nc.vector.reciprocal(out=scl[:], in_=scl[:])

            # ---- shared: feat_v for both samples [n2, 512] ----
            featv_ps = psum.tile([N2, DV], F32, tag="ps")
            for c in range(16):
                nc.tensor.matmul(
                    out=featv_ps[:],
                    lhsT=visf2_sb[:, c, :],
                    rhs=WvT_sb[:, c, :],
                    start=(c == 0),
                    stop=(c == 15),
                )
            featv_sb = singles.tile([N2, DV], F32)
            nc.vector.tensor_scalar_mul(out=featv_sb[:], in0=featv_ps[:], scalar1=scl[:])
            ftT2_ps = psum.tile([P, 4, N2], F32, tag="ps")
            for c in range(4):
                nc.tensor.matmul(
                    out=ftT2_ps[:, c, :],
                    lhsT=featv_sb[:, P * c : P * (c + 1)],
                    rhs=I128_sb[:],
                    start=(c == 0),
                    stop=(c == 3),
                )
            ftT2_sb = singles.tile([P, 4, N2], F32)
            nc.vector.tensor_copy(out=ftT2_sb[:], in_=ftT2_ps[:])

            # ---- shared: qT/u0T/u1T for both samples [d, 24] ----
            def lin_T(w_sb, x_ap, name):
                out_ps = psum.tile([P, 4, K2], F32, tag="ps", name=name + "_ps")
                for dc in range(4):
                    for wc in range(4):
                        nc.tensor.matmul(
                            out=out_ps[:, dc, :],
                            lhsT=w_sb[:, wc, P * dc : P * (dc + 1)],
                            rhs=x_ap[:, wc, :],
                            start=(dc == 0 and wc == 0),
                            stop=(dc == 3 and wc == 3),
                        )
                out_sb = singles.tile([P, 4, K2], F32, name=name)
                nc.scalar.copy(out=out_sb[:], in_=out_ps[:])
                return out_sb

            qT2_sb = lin_T(WnT_sb, nrep2, "qT2")
            u0T2_sb = lin_T(WA0_sb, rrep2, "u0T2")
            u1T2_sb = lin_T(WA1_sb, rrep2, "u1T2")

            # ---- per-sample pipeline ----
            for s in range(S):
                sm = smalls_sb[s]
                rmask = sm[:, _SM_RM : _SM_RM + N]
                bmmul = sm[:K, _SM_BM : _SM_BM + N]
                bmadd = sm[:K, _SM_BA : _SM_BA + N]
                GTm = sm[:K, _SM_GT : _SM_GT + R]
                STm = sm[:R, _SM_ST : _SM_ST + K]
                nmcol = sm[:K, _SM_NM : _SM_NM + 1]
                famul = sm[:1, _SM_FM : _SM_FM + N]
                faadd = sm[:1, _SM_FA : _SM_FA + N]
                ks = slice(K * s, K * (s + 1))
                ns = slice(N * s, N * (s + 1))

                # node attention
                logits_ps = psum.tile([K, N], F32, tag="ps")
                for c in range(4):
                    nc.tensor.matmul(
                        out=logits_ps[:],
                        lhsT=qT2_sb[:, c, ks],
                        rhs=ftT2_sb[:, c, ns],
                        start=(c == 0),
                        stop=(c == 3),
                    )
                lg_sb = ps.tile([K, N], F32)
                nc.vector.tensor_tensor(
                    out=lg_sb[:], in0=logits_ps[:], in1=bmmul, op=mybir.AluOpType.mult
                )
                nc.vector.tensor_tensor(
                    out=lg_sb[:], in0=lg_sb[:], in1=bmadd, op=mybir.AluOpType.add
                )
                nmx = ps.tile([K, 1], F32)
                nc.vector.tensor_reduce(
                    out=nmx[:], in_=lg_sb[:], axis=mybir.AxisListType.X,
                    op=mybir.AluOpType.max, negate=True,
                )
                e_sb = ps.tile([K, N], F32)
                ssum = ps.tile([K, 1], F32)
                nc.scalar.activation(
                    out=e_sb[:], in_=lg_sb[:],
                    func=mybir.ActivationFunctionType.Exp,
                    bias=nmx[:], scale=1.0, accum_out=ssum[:],
                )
                rs = ps.tile([K, 1], F32)
                nc.vector.reciprocal(out=rs[:], in_=ssum[:])
                nc.vector.tensor_tensor(
                    out=rs[:], in0=rs[:], in1=nmcol, op=mybir.AluOpType.mult
                )
                find_sb = ps.tile([K, N], F32)
                nc.vector.tensor_scalar_mul(out=find_sb[:], in0=e_sb[:], scalar1=rs[:])

                # g_findT; open find2T accumulation with findT
                gfT_ps = psum.tile([N, R], F32, tag="ps")
                nc.tensor.matmul(
                    out=gfT_ps[:], lhsT=find_sb[:], rhs=GTm, start=True, stop=True
                )
                gfT_sb = ps.tile([N, R], F32)
                nc.scalar.copy(out=gfT_sb[:], in_=gfT_ps[:])
                f2T_ps = psum.tile([N, K], F32, tag="ps")
                nc.tensor.matmul(
                    out=f2T_ps[:], lhsT=find_sb[:], rhs=I128_sb[:K, :K],
                    start=True, stop=False,
                )

                # A0 [r, n] and A1T [n, r]
                A0_ps = psum.tile([R, N], F32, tag="ps")
                for c in range(4):
                    nc.tensor.matmul(
                        out=A0_ps[:], lhsT=u0T2_sb[:, c, ks], rhs=ftT2_sb[:, c, ns],
                        start=(c == 0), stop=(c == 3),
                    )
                A0_sb = ps.tile([R, N], F32)
                nc.scalar.copy(out=A0_sb[:], in_=A0_ps[:])
                A1T_ps = psum.tile([N, R], F32, tag="ps")
                for c in range(4):
                    nc.tensor.matmul(
                        out=A1T_ps[:], lhsT=ftT2_sb[:, c, ns], rhs=u1T2_sb[:, c, ks],
                        start=(c == 0), stop=(c == 3),
                    )
                A1T_sb = ps.tile([N, R], F32)
                nc.scalar.copy(out=A1T_sb[:], in_=A1T_ps[:])

                # edge attention + h
                ea_all = ps.tile([N, R, N], F32)
                hT_ps = psum.tile([N, R], F32, tag="ps")
                for r in range(R):
                    B_ps = psum.tile([N, N], F32, tag="ps", name="B_ps")
                    nc.tensor.matmul(
                        out=B_ps[:],
                        lhsT=I128_sb[:K, r : r + 1].to_broadcast([K, N]),
                        rhs=A0_sb[:],
                        start=True, stop=True,
                    )
                    nc.scalar.activation(
                        out=ea_all[:, r, :], in_=B_ps[:],
                        func=mybir.ActivationFunctionType.Sigmoid,
                        bias=A1T_sb[:, r : r + 1], scale=1.0,
                    )
                nc.vector.tensor_tensor(
                    out=ea_all[:],
                    in0=ea_all[:],
                    in1=rmask[:, None, :].to_broadcast([N, R, N]),
                    op=mybir.AluOpType.mult,
                )
                for r in range(R):
                    nc.tensor.matmul(
                        out=hT_ps[:, r : r + 1],
                        lhsT=ea_all[:, r, :],
                        rhs=gfT_sb[:, r : r + 1],
                        start=(r == 0),
                        stop=(r == R - 1),
                    )
                hT_sb = ps.tile([N, R], F32)
                nc.scalar.copy(out=hT_sb[:], in_=hT_ps[:])
                h_ps = psum.tile([R, N], F32, tag="ps")
                nc.tensor.matmul(
                    out=h_ps[:], lhsT=hT_sb[:], rhs=I128_sb[:N, :N], start=True, stop=True
                )
                h_sb = ps.tile([R, N], F32)
                nc.scalar.copy(out=h_sb[:], in_=h_ps[:])

                # find2T = findT + h.T @ ST; final attention
                nc.tensor.matmul(
                    out=f2T_ps[:], lhsT=h_sb[:], rhs=STm, start=False, stop=True
                )
                fa_sb = ps.tile([N, 1], F32)
                nc.vector.tensor_reduce(
                    out=fa_sb[:], in_=f2T_ps[:], axis=mybir.AxisListType.X,
                    op=mybir.AluOpType.max,
                )
                faT_ps = psum.tile([1, N], F32, tag="ps")
                nc.tensor.matmul(
                    out=faT_ps[:], lhsT=fa_sb[:], rhs=I128_sb[:N, :N],
                    start=True, stop=True,
                )
                nr = ps.tile([1, 1], F32)
                nc.vector.tensor_reduce(
                    out=nr[:], in_=faT_ps[:], axis=mybir.AxisListType.X,
                    op=mybir.AluOpType.max,
                )
                nc.vector.tensor_scalar_max(out=nr[:], in0=nr[:], scalar1=1.0)
                nc.vector.reciprocal(out=nr[:], in_=nr[:])
                faT_sb = ps.tile([1, N], F32)
                nc.vector.tensor_scalar_mul(out=faT_sb[:], in0=faT_ps[:], scalar1=nr[:])
                nc.vector.tensor_tensor(
                    out=faT_sb[:], in0=faT_sb[:], in1=famul, op=mybir.AluOpType.mult
                )
                nc.vector.tensor_tensor(
                    out=faT_sb[:], in0=faT_sb[:], in1=faadd, op=mybir.AluOpType.add
                )

                # mem[:, s] = sum_n visf[:, :, n] * fa[n]
                fabc_ps = psum.tile([P, N], F32, tag="ps")
                nc.tensor.matmul(
                    out=fabc_ps[:], lhsT=ones_1xP[:], rhs=faT_sb[:],
                    start=True, stop=True,
                )
                wtmp = ps.tile([P, 16, N], F32, tag="wtmp")
                nc.vector.tensor_tensor(
                    out=wtmp[:],
                    in0=visf2_f[:, :, ns],
                    in1=fabc_ps[:, None, :].to_broadcast([P, 16, N]),
                    op=mybir.AluOpType.mult,
                )
                nc.vector.tensor_reduce(
                    out=mem2_sb[:, :, s], in_=wtmp[:], axis=mybir.AxisListType.X,
                    op=mybir.AluOpType.add,
                )

            # ---- W_out (data-parallel, f32r): out[2, 1024] ----
            WoT_sb = singles.tile([P, 16, DC], F32R if USE_F32R else F32)
            for g in range(4):
                nc.sync.dma_start(
                    out=WoT_sb[:, 4 * g : 4 * g + 4, :],
                    in_=d_WoT[:, 4 * g * DC : 4 * (g + 1) * DC].rearrange(
                        "p (o d) -> p o d", o=4
                    ),
                )
            bout_sb = singles.tile([S, DC], F32)
            nc.sync.dma_start(out=bout_sb[:], in_=d_bout[:])

            mem2r_sb = singles.tile([P, 16, S], F32R if USE_F32R else F32)
            nc.scalar.copy(out=mem2r_sb[:], in_=mem2_sb[:])
            out_sb = singles.tile([S, DC], F32)
            for h in range(2):
                o_ps = psum.tile([S, DC // 2], F32, tag="ps", name=f"o_ps{h}")
                for c in range(16):
                    nc.tensor.matmul(
                        out=o_ps[:],
                        lhsT=mem2r_sb[:, c, :],
                        rhs=WoT_sb[:, c, (DC // 2) * h : (DC // 2) * (h + 1)],
                        start=(c == 0),
                        stop=(c == 15),
                    )
                nc.vector.tensor_tensor(
                    out=out_sb[:, (DC // 2) * h : (DC // 2) * (h + 1)],
                    in0=o_ps[:],
                    in1=bout_sb[:, (DC // 2) * h : (DC // 2) * (h + 1)],
                    op=mybir.AluOpType.add,
                )
            nc.sync.dma_start(out=d_out[:], in_=out_sb[:])

    nc.finalize()
    return nc


def _host_prep(inputs):
    node_rep = np.asarray(inputs["node_rep"], np.float32)
    relate_rep = np.asarray(inputs["relate_rep"], np.float32)
    relate_os = np.asarray(inputs["relate_os"])
    relate_mask = np.asarray(inputs["relate_mask"], np.float32)
    vision_feat = np.asarray(inputs["vision_feat"], np.float32)
    relation_mask = np.asarray(inputs["relation_mask"], np.float32)
    box_mask = np.asarray(inputs["box_mask"], np.float32)
    node_mask = np.asarray(inputs["node_mask"], np.float32)
    norm_w = np.asarray(inputs["norm_w"], np.float32)
    W_v = np.asarray(inputs["W_v"], np.float32)
    W_e = np.asarray(inputs["W_e"], np.float32)
    W_node = np.asarray(inputs["W_node"], np.float32)
    W_rel = np.asarray(inputs["W_rel"], np.float32)
    W_out = np.asarray(inputs["W_out"], np.float32)
    b_out = np.asarray(inputs["b_out"], np.float32)

    s_mean = np.float32(np.mean(norm_w))
    WvT = (W_v.T * s_mean).astype(np.float32)
    WnT = (W_node.T / np.float32(np.sqrt(DV))).astype(np.float32)
    WA0 = (W_rel.T @ W_e[:, :DV] / np.float32(np.sqrt(DE))).astype(np.float32)
    WA1 = (W_rel.T @ W_e[:, DV:] / np.float32(np.sqrt(DE))).astype(np.float32)
    WoT = np.ascontiguousarray(W_out.T)

    subj = relate_os[..., 1].astype(np.int64)
    obj = relate_os[..., 0].astype(np.int64)
    valid = (subj != -1).astype(np.float32)
    obj_c = np.clip(obj, 0, K - 1)
    subj_c = np.clip(subj, 0, K - 1)
    G = np.zeros((B, R, K), np.float32)
    STm = np.zeros((B, R, K), np.float32)
    bi = np.arange(B)[:, None]
    ri = np.arange(R)[None, :]
    G[bi, ri, obj_c] = valid * relate_mask
    STm[bi, ri, subj_c] = 1.0

    bmmul = (box_mask > 0).astype(np.float32)
    bmadd = (bmmul - 1.0) * np.float32(1e9)
    famul = box_mask
    faadd = (1.0 - box_mask) * np.float32(1e-7)

    WvT_p = _pack(WvT)
    WnT_p = _pack(WnT)
    WA0_p = _pack(WA0)
    WA1_p = _pack(WA1)
    WoT_p = _pack(WoT)
    I128 = np.eye(P, dtype=np.float32)
    bout2 = np.ascontiguousarray(np.broadcast_to(b_out[None, :], (S, DC))).astype(
        np.float32
    )

    def smalls_for(b):
        sm = np.zeros((N, SMALLS_F), np.float32)
        sm[:, _SM_RM : _SM_RM + N] = relation_mask[b]
        sm[:K, _SM_BM : _SM_BM + N] = bmmul[b][None, :]
        sm[:K, _SM_BA : _SM_BA + N] = bmadd[b][None, :]
        sm[:K, _SM_GT : _SM_GT + R] = G[b].T
        sm[:R, _SM_ST : _SM_ST + K] = STm[b]
        sm[:K, _SM_NM] = node_mask[b]
        sm[0, _SM_FM : _SM_FM + N] = famul[b]
        sm[0, _SM_FA : _SM_FA + N] = faadd[b]
        return sm

    in_maps = []
    for c in range(NCORES):
        b0 = S * c
        visf2 = np.concatenate(
            [_pack(vision_feat[b]).reshape(P, 16, N) for b in range(b0, b0 + S)],
            axis=2,
        ).reshape(P, -1)
        nrep2 = np.concatenate(
            [
                _pack(np.ascontiguousarray(node_rep[b].T)).reshape(P, 4, K)
                for b in range(b0, b0 + S)
            ],
            axis=2,
        ).reshape(P, -1)
        rrep2 = np.concatenate(
            [
                _pack(np.ascontiguousarray(relate_rep[b].T)).reshape(P, 4, R)
                for b in range(b0, b0 + S)
            ],
            axis=2,
        ).reshape(P, -1)
        m = {
            "visf2": np.ascontiguousarray(visf2),
            "reps": np.ascontiguousarray(np.concatenate([nrep2, rrep2], axis=1)),
            "WvT": WvT_p,
            "WnT": WnT_p,
            "WA0": WA0_p,
            "WA1": WA1_p,
            "WoT": WoT_p,
            "bout": bout2,
            "smalls": np.stack([smalls_for(b) for b in range(b0, b0 + S)]),
            "I128": I128,
        }
        in_maps.append(m)
    return in_maps


def kernel(**inputs) -> np.ndarray:
    if "nc" not in _cache:
        _cache["nc"] = build_nc()
    nc = _cache["nc"]
    in_maps = _host_prep(inputs)
    res = run_bass_kernel_spmd(nc, in_maps, core_ids=list(range(NCORES)))
    outs = [res.results[c]["out"] for c in range(NCORES)]
    return np.concatenate(outs, axis=0).astype(np.float32)
